# revision 8
# baseline (speedup 1.0000x reference)
"""Trainium2 Bass kernel for nn_ConvolutionalSelfAttention.

The reference network collapses algebraically. Per image b (Xt = batch[b]
viewed [C, HW], c-major):
  K_t = key_w @ Xt + key_b          [C, HW]
  Q_t = query_w @ Xt + query_b      [C, HW]
  v   = value_w @ Xt + value_b      [HW]
  rk[n] = 1/||K_t[:,n]||, rq[m] = 1/||Q_t[:,m]||
  E[n,m] = exp(rk[n] rq[m] (K_t[:,n] . Q_t[:,m]))       (full 1024x1024 Gram)
  V[m] = (sum_n v[n] E[n,m]) / (sum_n E[n,m])
  P[c,m] = Xt[c,m] V[m]
  out[b,c,i,j] = 3x3 valid box-sum of P over the spatial grid

Sharding: data-parallel over batch (16 images over 8 cores, 2 each).

v2 design:
- host ships x / weights in bf16 (half the input DMA, feeds bf16 PE ops)
- Gram in fp8e4 with MatmulPerfMode.DoubleRow (0.5 cyc/col, K=256 one pass)
- exp on ACT -> es bf16; numer/denom via one [v;1]-stationary matmul chunk
- rq/V broadcast via PE ones-matmul; qn8 = (ppQ+bq)*rq_bc fused on DVE
- 3x3 box-sum fully on DVE/Pool in bf16 (DVE 4x mode), output written with
  a casting gpsimd DMA (bf16 -> f32), freeing PE + PSUM in the tail
- rk / v transposed to partition form via small DRAM bounces (off critical
  path)
"""
import os
import numpy as np

os.environ.setdefault("BASS_NEVER_TRACE", "1")

import contextlib

import concourse.bass as bass
import concourse.bacc as bacc
import concourse.tile as tile
from concourse import mybir
from concourse.bass_utils import run_bass_kernel_spmd

F32 = mybir.dt.float32
F32R = mybir.dt.float32r
BF16 = mybir.dt.bfloat16
F8 = mybir.dt.float8e4
AF = mybir.ActivationFunctionType
OP = mybir.AluOpType
DR = mybir.MatmulPerfMode.DoubleRow

B, C, H, W = 16, 256, 32, 32
HW = H * W            # 1024
CH = CW = 30
NF = CH * CW          # 900
NCORES = 8
BL = B // NCORES      # images per core
NCH = C // 128        # channel chunks
NNJ = HW // 128       # row blocks of the gram


def _ap(t, extra_off, pattern):
    return bass.AP(tensor=t.tensor, offset=t.offset + extra_off,
                   ap=[list(x) for x in pattern])


def build_program():
    nc = bacc.Bacc("TRN2", target_bir_lowering=False, debug=False,
                   num_devices=NCORES)
    # walrus's lower_act places activation-table loads; bacc's pre-placed
    # loads produce NEFFs this runtime refuses to load.
    nc.insert_act_table_loads = lambda: None

    def din(name, shape, dt):
        return nc.dram_tensor(name, list(shape), dt, kind="ExternalInput").ap()

    x_d = din("x", (BL, C, HW), BF16)
    wall_d = din("wall", (C, 2 * C + 2), BF16)  # [query_w.T | key_w.T | wv | 0]
    ball_d = din("ball", (128, 2 * NCH), F32)   # [bk | bq]
    bv_d = din("bv", (1, 2), F32)
    ones1_d = din("ones1", (1, 128), F32R)

    out_d = nc.dram_tensor("out", [BL, C, NF], F32, kind="ExternalOutput").ap()

    with tile.TileContext(nc) as tc:
        with contextlib.ExitStack() as ctx:
            consts = ctx.enter_context(tc.tile_pool(name="consts", bufs=1))
            sbuf = ctx.enter_context(tc.tile_pool(name="sbuf", bufs=2))
            epool = ctx.enter_context(tc.tile_pool(name="epool", bufs=4))
            pgp = ctx.enter_context(tc.tile_pool(name="pgp", bufs=2, space="PSUM"))
            unp = ctx.enter_context(tc.tile_pool(name="unp", bufs=2, space="PSUM"))
            dramp = ctx.enter_context(tc.tile_pool(name="dramp", bufs=2, space="DRAM"))

            # ---------------- constants ----------------
            wall_t = consts.tile([128, NCH, 2 * C + 2], BF16, tag="wall", name="wall")
            wallv = wall_d.rearrange("(kc p) m -> p kc m", p=128)
            nc.sync.dma_start(out=wall_t[:, :, 0:C], in_=wallv[:, :, 0:C])
            wq_t = wall_t[:, :, 0:C]
            wk_t = wall_t[:, :, C:2 * C]
            wv_t = wall_t[:, :, 2 * C:2 * C + 1]
            ball_t = consts.tile([128, 2 * NCH], F32, tag="ball", name="ball")
            bk_t = ball_t[:, 0:NCH]
            bq_t = ball_t[:, NCH:2 * NCH]
            bv_t = consts.tile([128, 1], F32, tag="bv", name="bv")
            ones_b = consts.tile([128, 1], BF16, tag="ones_b", name="ones_b")
            nc.vector.memset(ones_b, 1.0)
            ones1 = consts.tile([1, 128], F32R, tag="ones1", name="ones1")
            nc.sync.dma_start(out=ones1, in_=ones1_d)

            def load_rest_consts():
                nc.scalar.dma_start(out=wall_t[:, :, C:], in_=wallv[:, :, C:])
                nc.sync.dma_start(out=ball_t, in_=ball_d)
                nc.sync.dma_start(out=bv_t, in_=_ap(bv_d, 0, [[0, 128], [1, 1]]))

            def warmup():
                pw = pgp.tile([128, HW], F32, tag="pg", name="warm")
                for _ in range(30):
                    nc.tensor.matmul(pw[0:1, 0:1], ones_b, ones_b,
                                     start=True, stop=True)

            # ---------------- per-b state ----------------
            xs, kn8s, qn8s, rkts, v1s = {}, {}, {}, {}, {}
            pnds, Vbcs = {}, {}

            def load_x(b):
                xs[b] = sbuf.tile([128, NCH, HW], BF16, tag="x", name="x")
                xv = x_d[b].rearrange("(kc p) m -> p kc m", p=128)
                engs = [nc.sync, nc.scalar]
                for kc in range(NCH):
                    engs[kc].dma_start(out=xs[b][:, kc, :], in_=xv[:, kc, :])

            def mm_proj(psum, w_t, mc, b):
                for kc in range(NCH):
                    for nt in range(2):
                        nc.tensor.matmul(
                            psum[:, nt * 512:(nt + 1) * 512],
                            w_t[:, kc, mc * 128:(mc + 1) * 128],
                            xs[b][:, kc, nt * 512:(nt + 1) * 512],
                            start=(kc == 0), stop=(kc == NCH - 1))

            def proj(b):
                # ---- Q projections ----
                sqq = sbuf.tile([128, NCH, HW], BF16, tag="sqq", name="sqq")
                ppqs = []
                for mc in range(NCH):
                    pp = pgp.tile([128, HW], F32, tag="pg", name="ppq")
                    ppqs.append(pp)
                    mm_proj(pp, wq_t, mc, b)
                    # sqQ = (pp + bq)^2 on ACT
                    nc.scalar.activation(
                        sqq[:, mc, :], pp, AF.Square, bias=bq_t[:, mc:mc + 1])
                # ssqQ -> pssq row 0
                pssq = unp.tile([128, HW], F32, tag="un", name="pssq")
                for nt in range(2):
                    for kc in range(NCH):
                        nc.tensor.matmul(
                            pssq[0:1, nt * 512:(nt + 1) * 512], ones_b,
                            sqq[:, kc, nt * 512:(nt + 1) * 512],
                            start=(kc == 0), stop=(kc == NCH - 1))
                # rq chain: sqrt(x/256) (ACT) -> recip (DVE, f32r out) = 16/||Q||
                rr = sbuf.tile([1, HW], F32, tag="rr", name="rr")
                nc.scalar.activation(rr, pssq[0:1, :], AF.Sqrt, scale=1.0 / 256.0)
                rq1 = sbuf.tile([1, HW], F32R, tag="rq1", name="rq1")
                with nc.allow_low_precision(reason="f32r has 13 mantissa bits; plenty for rq"):
                    nc.vector.reciprocal(rq1, rr)
                rq_bc = unp.tile([128, HW], F32, tag="un", name="rq_bc")
                for nt in range(2):
                    nc.tensor.matmul(
                        rq_bc[:, nt * 512:(nt + 1) * 512], ones1,
                        rq1[:, nt * 512:(nt + 1) * 512], start=True, stop=True)

                # ---- v projection ----
                pv = pgp.tile([128, HW], F32, tag="pg", name="pv")
                for nt in range(2):
                    for kc in range(NCH):
                        nc.tensor.matmul(
                            pv[0:1, nt * 512:(nt + 1) * 512], wv_t[:, kc, :],
                            xs[b][:, kc, nt * 512:(nt + 1) * 512],
                            start=(kc == 0), stop=(kc == NCH - 1))
                v_sb = sbuf.tile([1, HW], F32, tag="v_sb", name="v_sb")
                nc.vector.tensor_scalar_add(v_sb, pv[0:1, :], bv_t[0:1, 0:1])
                d_v = dramp.tile([1, HW], F32, tag="d_v", name="d_v")
                nc.sync.dma_start(out=d_v, in_=v_sb)
                vt = sbuf.tile([128, NNJ], F32, tag="vt", name="vt")
                nc.sync.dma_start(out=vt, in_=_ap(d_v, 0, [[1, 128], [128, NNJ]]))
                v1 = sbuf.tile([128, NNJ, 33], BF16, tag="v1", name="v1")
                v1s[b] = v1
                nc.gpsimd.memset(v1, 0.0)
                nc.gpsimd.memset(v1[:, :, 32], 1.0)
                nc.gpsimd.tensor_copy(v1[:, :, 0], vt)

                # ---- K projections ----
                kn8 = sbuf.tile([128, NCH, HW], F8, tag="kn8", name="kn8")
                kn8s[b] = kn8
                sqk = sbuf.tile([128, NCH, HW], BF16, tag="sqk", name="sqk")
                for mc in range(NCH):
                    pp = pgp.tile([128, HW], F32, tag="pg", name="ppk")
                    mm_proj(pp, wk_t, mc, b)
                    nc.vector.tensor_scalar_add(kn8[:, mc, :], pp, bk_t[:, mc:mc + 1])
                    nc.scalar.activation(
                        sqk[:, mc, :], pp, AF.Square, bias=bk_t[:, mc:mc + 1])
                # ssqK -> pssq row 32
                for nt in range(2):
                    for kc in range(NCH):
                        nc.tensor.matmul(
                            pssq[32:33, nt * 512:(nt + 1) * 512], ones_b,
                            sqk[:, kc, nt * 512:(nt + 1) * 512],
                            start=(kc == 0), stop=(kc == NCH - 1))
                # rk chain: recip -> sqrt(x/256) = rk/16, bounce to [128, NNJ]
                rrk = sbuf.tile([1, HW], F32, tag="rrk", name="rrk")
                nc.vector.reciprocal(rrk, pssq[32:33, :])
                rk_row = sbuf.tile([1, HW], F32, tag="rk_row", name="rk_row")
                nc.scalar.activation(rk_row, rrk, AF.Sqrt, scale=1.0 / 256.0)
                d_rk = dramp.tile([1, HW], F32, tag="d_rk", name="d_rk")
                nc.sync.dma_start(out=d_rk, in_=rk_row)
                rkt = sbuf.tile([128, NNJ], F32, tag="rkt", name="rkt")
                rkts[b] = rkt
                nc.sync.dma_start(out=rkt, in_=_ap(d_rk, 0, [[1, 128], [128, NNJ]]))

                # ---- qn8 = (ppQ + bq) * rq_sb -> fp8 (fused on DVE) ----
                rq_sb = sbuf.tile([128, HW], BF16, tag="rq_sb", name="rq_sb")
                nc.vector.tensor_copy(rq_sb, rq_bc)
                qn8 = sbuf.tile([128, NCH, HW], F8, tag="qn8", name="qn8")
                qn8s[b] = qn8
                for mc in range(NCH):
                    nc.vector.scalar_tensor_tensor(
                        qn8[:, mc, :], ppqs[mc], bq_t[:, mc:mc + 1], rq_sb,
                        op0=OP.add, op1=OP.mult)

            def gram(b):
                pnd = unp.tile([128, HW], F32, tag="un", name="pnd")
                pnds[b] = pnd
                pgs, es = [None] * NNJ, [None] * NNJ

                def gram_chunk(nj):
                    pg = pgp.tile([128, HW], F32, tag="pg", name="pg")
                    pgs[nj] = pg
                    for nt in range(2):
                        nc.tensor.matmul(
                            pg[:, nt * 512:(nt + 1) * 512],
                            kn8s[b][:, :, nj * 128:(nj + 1) * 128],
                            qn8s[b][:, :, nt * 512:(nt + 1) * 512],
                            start=True, stop=True, perf_mode=DR)

                def exp_chunk(nj):
                    e = epool.tile([128, HW], BF16, tag="e", name="e")
                    es[nj] = e
                    nc.scalar.activation(
                        e, pgs[nj], AF.Exp, scale=rkts[b][:, nj:nj + 1])

                def numer_chunk(nj):
                    for nt in range(2):
                        nc.tensor.matmul(
                            pnd[0:33, nt * 512:(nt + 1) * 512],
                            v1s[b][:, nj, :],
                            es[nj][:, nt * 512:(nt + 1) * 512],
                            start=(nj == 0), stop=(nj == NNJ - 1))

                gram_chunk(0)
                exp_chunk(0)
                for nj in range(1, NNJ):
                    gram_chunk(nj)
                    exp_chunk(nj)
                    numer_chunk(nj - 1)
                numer_chunk(NNJ - 1)

            def vchain(b):
                # V row = pnd[0]/pnd[1]; broadcast via PE ones matmul
                rd = sbuf.tile([1, HW], F32, tag="rd", name="rd")
                nc.vector.reciprocal(rd, pnds[b][32:33, :])
                V1 = sbuf.tile([1, HW], F32R, tag="V1", name="V1")
                with nc.allow_low_precision(reason="f32r has 13 mantissa bits; plenty for V"):
                    nc.vector.tensor_mul(V1, pnds[b][0:1, :], rd)
                V_bc = unp.tile([128, HW], F32, tag="un", name="V_bc")
                Vbcs[b] = V_bc
                for nt in range(2):
                    nc.tensor.matmul(
                        V_bc[:, nt * 512:(nt + 1) * 512], ones1,
                        V1[:, nt * 512:(nt + 1) * 512], start=True, stop=True)

            def conv(b):
                # V_bc PSUM -> SBUF bf16 once, then all-bf16 4x DVE + Pool tail
                V_sb = sbuf.tile([128, HW], BF16, tag="V_sb", name="V_sb")
                nc.vector.tensor_copy(V_sb, Vbcs[b])
                for mc in range(NCH):
                    p_sb = sbuf.tile([128, H, W], BF16, tag="p_sb", name="p_sb")
                    nc.vector.tensor_mul(
                        p_sb.rearrange("p h w -> p (h w)"), xs[b][:, mc, :], V_sb)
                    t1 = sbuf.tile([128, H, 31], BF16, tag="t1", name="t1")
                    nc.vector.tensor_add(t1, p_sb[:, :, 0:31], p_sb[:, :, 1:32])
                    s1 = sbuf.tile([128, H, CW], BF16, tag="s1", name="s1")
                    nc.vector.tensor_add(s1, t1[:, :, 0:CW], p_sb[:, :, 2:32])
                    o1 = sbuf.tile([128, CH, CW], BF16, tag="o1", name="o1")
                    nc.vector.tensor_add(o1, s1[:, 0:CH, :], s1[:, 1:CH + 1, :])
                    outb = sbuf.tile([128, NF], BF16, tag="outb", name="outb")
                    nc.vector.tensor_add(
                        outb.rearrange("p (h w) -> p h w", h=CH),
                        o1, s1[:, 2:CH + 2, :])
                    # casting DMA bf16 -> f32 (gpsimd SWDGE)
                    nc.gpsimd.dma_start(
                        out=out_d[b, mc * 128:(mc + 1) * 128, :], in_=outb)

            # ---------------- emission schedule ----------------
            load_x(0)
            load_rest_consts()
            load_x(1)
            warmup()
            proj(0)
            proj(1)
            gram(0)
            vchain(0)
            gram(1)
            conv(0)
            vchain(1)
            conv(1)

    nc.compile()
    return nc


_CACHE = {}


def _get_program():
    if "nc" not in _CACHE:
        _CACHE["nc"] = build_program()
    return _CACHE["nc"]


def make_in_maps(batch, key_w, key_b, query_w, query_b, value_w, value_b):
    import ml_dtypes
    bf = ml_dtypes.bfloat16
    wall = np.zeros((C, 2 * C + 2), np.float32)
    wall[:, 0:C] = query_w.T
    wall[:, C:2 * C] = key_w.T
    wall[:, 2 * C] = value_w[0]
    wall = wall.astype(bf)
    ball = np.zeros((128, 2 * NCH), np.float32)
    ball[:, 0:NCH] = key_b.reshape(NCH, 128).T
    ball[:, NCH:2 * NCH] = query_b.reshape(NCH, 128).T
    bv = np.zeros((1, 2), np.float32)
    bv[0, 0] = value_b[0]
    in_maps = []
    for i in range(NCORES):
        xb = batch[i * BL:(i + 1) * BL].reshape(BL, C, HW).astype(bf)
        in_maps.append({
            "x": np.ascontiguousarray(xb),
            "wall": wall, "ball": ball, "bv": bv,
            "ones1": np.ones((1, 128), np.float32),
        })
    return in_maps


def kernel(batch, key_w, key_b, query_w, query_b, value_w, value_b,
           local_indices=None, **_ignored):
    batch = np.ascontiguousarray(np.asarray(batch, np.float32))
    args = [np.asarray(a, np.float32) for a in
            (key_w, key_b, query_w, query_b, value_w, value_b)]
    nc = _get_program()
    in_maps = make_in_maps(batch, *args)
    res = run_bass_kernel_spmd(nc, in_maps, list(range(NCORES)))
    outs = [np.asarray(r["out"], np.float32) for r in res.results]
    return np.concatenate(outs, axis=0).reshape(B, C, CH, CW)


# revision 12
# speedup vs baseline: 1.3440x; 1.3440x over previous
"""Trainium2 Bass kernel for nn_ConvolutionalSelfAttention.

The reference network collapses algebraically. Per image b (Xt = batch[b]
viewed [C, HW], c-major):
  K_t = key_w @ Xt + key_b          [C, HW]
  Q_t = query_w @ Xt + query_b      [C, HW]
  v   = value_w @ Xt + value_b      [HW]
  rk[n] = 1/||K_t[:,n]||, rq[m] = 1/||Q_t[:,m]||
  E[n,m] = exp(rk[n] rq[m] (K_t[:,n] . Q_t[:,m]))       (full 1024x1024 Gram)
  V[m] = (sum_n v[n] E[n,m]) / (sum_n E[n,m])
  P[c,m] = Xt[c,m] V[m]
  out[b,c,i,j] = 3x3 valid box-sum of P over the spatial grid

Sharding: data-parallel over batch (16 images over 8 cores, 2 each).

v2 design:
- host ships x / weights in bf16 (half the input DMA, feeds bf16 PE ops)
- Gram in fp8e4 with MatmulPerfMode.DoubleRow (0.5 cyc/col, K=256 one pass)
- exp on ACT -> es bf16; numer/denom via one [v;1]-stationary matmul chunk
- rq/V broadcast via PE ones-matmul; qn8 = (ppQ+bq)*rq_bc fused on DVE
- 3x3 box-sum fully on DVE/Pool in bf16 (DVE 4x mode), output written with
  a casting gpsimd DMA (bf16 -> f32), freeing PE + PSUM in the tail
- rk / v transposed to partition form via small DRAM bounces (off critical
  path)
"""
import os
import numpy as np

os.environ.setdefault("BASS_NEVER_TRACE", "1")

import contextlib

import concourse.bass as bass
import concourse.bacc as bacc
import concourse.tile as tile
from concourse import mybir
from concourse.bass_utils import run_bass_kernel_spmd

F32 = mybir.dt.float32
F32R = mybir.dt.float32r
BF16 = mybir.dt.bfloat16
F8 = mybir.dt.float8e4
AF = mybir.ActivationFunctionType
OP = mybir.AluOpType
DR = mybir.MatmulPerfMode.DoubleRow

B, C, H, W = 16, 256, 32, 32
HW = H * W            # 1024
CH = CW = 30
NF = CH * CW          # 900
NCORES = 8
BL = B // NCORES      # images per core
NCH = C // 128        # channel chunks
NNJ = HW // 128       # row blocks of the gram


def _ap(t, extra_off, pattern):
    return bass.AP(tensor=t.tensor, offset=t.offset + extra_off,
                   ap=[list(x) for x in pattern])


def build_program():
    nc = bacc.Bacc("TRN2", target_bir_lowering=False, debug=False,
                   num_devices=NCORES)
    # walrus's lower_act places activation-table loads; bacc's pre-placed
    # loads produce NEFFs this runtime refuses to load.
    nc.insert_act_table_loads = lambda: None

    def din(name, shape, dt):
        return nc.dram_tensor(name, list(shape), dt, kind="ExternalInput").ap()

    x_d = din("x", (BL, C, HW), BF16)
    wall_d = din("wall", (C, 2 * C + 2), BF16)  # [query_w.T | key_w.T | wv | 0]
    ball_d = din("ball", (128, 2 * NCH), F32)   # [bk | bq]
    bv_d = din("bv", (1, 2), F32)

    out_d = nc.dram_tensor("out", [BL, C, NF], F32, kind="ExternalOutput").ap()

    with tile.TileContext(nc) as tc:
        with contextlib.ExitStack() as ctx:
            consts = ctx.enter_context(tc.tile_pool(name="consts", bufs=1))
            sbuf = ctx.enter_context(tc.tile_pool(name="sbuf", bufs=2))
            epool = ctx.enter_context(tc.tile_pool(name="epool", bufs=4))
            pgp = ctx.enter_context(tc.tile_pool(name="pgp", bufs=2, space="PSUM"))
            unp = ctx.enter_context(tc.tile_pool(name="unp", bufs=2, space="PSUM"))
            dramp = ctx.enter_context(tc.tile_pool(name="dramp", bufs=2, space="DRAM"))

            # ---------------- constants ----------------
            wall_t = consts.tile([128, NCH, 2 * C + 2], BF16, tag="wall", name="wall")
            wallv = wall_d.rearrange("(kc p) m -> p kc m", p=128)
            nc.sync.dma_start(out=wall_t[:, :, 0:C], in_=wallv[:, :, 0:C])
            wq_t = wall_t[:, :, 0:C]
            wk_t = wall_t[:, :, C:2 * C]
            wv_t = wall_t[:, :, 2 * C:2 * C + 1]
            ball_t = consts.tile([128, 2 * NCH], F32, tag="ball", name="ball")
            bk_t = ball_t[:, 0:NCH]
            bq_t = ball_t[:, NCH:2 * NCH]
            bv_t = consts.tile([128, 1], F32, tag="bv", name="bv")
            ones_b = consts.tile([128, 1], BF16, tag="ones_b", name="ones_b")
            nc.vector.memset(ones_b, 1.0)
            ones1 = consts.tile([1, 128], BF16, tag="ones1", name="ones1")
            nc.vector.memset(ones1, 1.0)
            lnc = consts.tile([1, 2], F32, tag="lnc", name="lnc")
            nc.vector.memset(lnc[:, 0:1], float(np.log(16.0)))
            nc.vector.memset(lnc[:, 1:2], float(-np.log(16.0)))

            def load_rest_consts():
                nc.scalar.dma_start(out=wall_t[:, :, C:], in_=wallv[:, :, C:])
                nc.sync.dma_start(out=ball_t, in_=ball_d)
                nc.sync.dma_start(out=bv_t, in_=_ap(bv_d, 0, [[0, 128], [1, 1]]))

            def warmup():
                pw = pgp.tile([128, HW], F32, tag="pg", name="warm")
                for _ in range(30):
                    nc.tensor.matmul(pw[0:1, 0:1], ones_b, ones_b,
                                     start=True, stop=True)

            # ---------------- per-b state ----------------
            xs, kn8s, qn8s, rkts, v1s = {}, {}, {}, {}, {}
            pnds, Vbcs = {}, {}

            def load_x(b):
                xs[b] = sbuf.tile([128, NCH, HW], BF16, tag="x", name="x")
                xv = x_d[b].rearrange("(kc p) m -> p kc m", p=128)
                engs = [nc.sync, nc.scalar]
                for kc in range(NCH):
                    engs[kc].dma_start(out=xs[b][:, kc, :], in_=xv[:, kc, :])

            def mm_proj(psum, w_t, mc, b):
                for kc in range(NCH):
                    for nt in range(2):
                        nc.tensor.matmul(
                            psum[:, nt * 512:(nt + 1) * 512],
                            w_t[:, kc, mc * 128:(mc + 1) * 128],
                            xs[b][:, kc, nt * 512:(nt + 1) * 512],
                            start=(kc == 0), stop=(kc == NCH - 1))

            def proj(b):
                # ---- Q projections ----
                sqq = sbuf.tile([128, NCH, HW], BF16, tag="sqq", name="sqq")
                ppqs = []
                for mc in range(NCH):
                    pp = pgp.tile([128, HW], F32, tag="pg", name="ppq")
                    ppqs.append(pp)
                    mm_proj(pp, wq_t, mc, b)
                    # sqQ = (pp + bq)^2 on ACT
                    nc.scalar.activation(
                        sqq[:, mc, :], pp, AF.Square, bias=bq_t[:, mc:mc + 1])
                # ssqQ -> pssq row 0
                pssq = unp.tile([128, HW], F32, tag="un", name="pssq")
                for nt in range(2):
                    for kc in range(NCH):
                        nc.tensor.matmul(
                            pssq[0:1, nt * 512:(nt + 1) * 512], ones_b,
                            sqq[:, kc, nt * 512:(nt + 1) * 512],
                            start=(kc == 0), stop=(kc == NCH - 1))
                # rq chain: exp(-0.5*ln(ssq) + ln16) = 16/||Q|| (ACT only)
                rr = sbuf.tile([1, HW], F32, tag="rr", name="rr")
                nc.scalar.activation(rr, pssq[0:1, :], AF.Ln)
                rq1 = sbuf.tile([1, HW], BF16, tag="rq1", name="rq1")
                nc.scalar.activation(rq1, rr, AF.Exp, scale=-0.5,
                                     bias=lnc[0:1, 0:1])
                rq_bc = unp.tile([128, HW], F32, tag="un", name="rq_bc")
                for nt in range(2):
                    nc.tensor.matmul(
                        rq_bc[:, nt * 512:(nt + 1) * 512], ones1,
                        rq1[:, nt * 512:(nt + 1) * 512], start=True, stop=True)

                # ---- v projection ----
                pv = pgp.tile([128, HW], F32, tag="pg", name="pv")
                for nt in range(2):
                    for kc in range(NCH):
                        nc.tensor.matmul(
                            pv[0:1, nt * 512:(nt + 1) * 512], wv_t[:, kc, :],
                            xs[b][:, kc, nt * 512:(nt + 1) * 512],
                            start=(kc == 0), stop=(kc == NCH - 1))
                v_sb = sbuf.tile([1, HW], F32, tag="v_sb", name="v_sb")
                nc.vector.tensor_scalar_add(v_sb, pv[0:1, :], bv_t[0:1, 0:1])
                d_v = dramp.tile([1, HW], F32, tag="d_v", name="d_v")
                nc.sync.dma_start(out=d_v, in_=v_sb)
                vt = sbuf.tile([128, NNJ], F32, tag="vt", name="vt")
                nc.sync.dma_start(out=vt, in_=_ap(d_v, 0, [[1, 128], [128, NNJ]]))
                v1 = sbuf.tile([128, NNJ, 33], BF16, tag="v1", name="v1")
                v1s[b] = v1
                nc.gpsimd.memset(v1, 0.0)
                nc.gpsimd.memset(v1[:, :, 32], 1.0)
                nc.gpsimd.tensor_copy(v1[:, :, 0], vt)

                # ---- K projections ----
                kn8 = sbuf.tile([128, NCH, HW], F8, tag="kn8", name="kn8")
                kn8s[b] = kn8
                sqk = sbuf.tile([128, NCH, HW], BF16, tag="sqk", name="sqk")
                for mc in range(NCH):
                    pp = pgp.tile([128, HW], F32, tag="pg", name="ppk")
                    mm_proj(pp, wk_t, mc, b)
                    nc.vector.tensor_scalar_add(kn8[:, mc, :], pp, bk_t[:, mc:mc + 1])
                    nc.vector.tensor_mul(sqk[:, mc, :], kn8[:, mc, :], kn8[:, mc, :])
                # ssqK -> pssq row 32
                for nt in range(2):
                    for kc in range(NCH):
                        nc.tensor.matmul(
                            pssq[32:33, nt * 512:(nt + 1) * 512], ones_b,
                            sqk[:, kc, nt * 512:(nt + 1) * 512],
                            start=(kc == 0), stop=(kc == NCH - 1))
                # rk chain: exp(-0.5*ln(ssq) - ln16) = rk/16, bounce to [128, NNJ]
                rrk = sbuf.tile([1, HW], F32, tag="rrk", name="rrk")
                nc.scalar.activation(rrk, pssq[32:33, :], AF.Ln)
                rk_row = sbuf.tile([1, HW], F32, tag="rk_row", name="rk_row")
                nc.scalar.activation(rk_row, rrk, AF.Exp, scale=-0.5,
                                     bias=lnc[0:1, 1:2])
                d_rk = dramp.tile([1, HW], F32, tag="d_rk", name="d_rk")
                nc.sync.dma_start(out=d_rk, in_=rk_row)
                rkt = sbuf.tile([128, NNJ], F32, tag="rkt", name="rkt")
                rkts[b] = rkt
                nc.sync.dma_start(out=rkt, in_=_ap(d_rk, 0, [[1, 128], [128, NNJ]]))

                # ---- qn8 = (ppQ + bq) * rq_sb -> fp8 (fused on DVE) ----
                rq_sb = sbuf.tile([128, HW], BF16, tag="rq_sb", name="rq_sb")
                nc.vector.tensor_copy(rq_sb, rq_bc)
                qn8 = sbuf.tile([128, NCH, HW], F8, tag="qn8", name="qn8")
                qn8s[b] = qn8
                for mc in range(NCH):
                    nc.vector.scalar_tensor_tensor(
                        qn8[:, mc, :], ppqs[mc], bq_t[:, mc:mc + 1], rq_sb,
                        op0=OP.add, op1=OP.mult)

            def gram(b, mid_mm=None):
                pnd = unp.tile([128, HW], F32, tag="un", name="pnd")
                pnds[b] = pnd
                pgs, es = [None] * NNJ, [None] * NNJ

                def gram_chunk(nj):
                    pg = pgp.tile([128, HW], F32, tag="pg", name="pg")
                    pgs[nj] = pg
                    for nt in range(2):
                        nc.tensor.matmul(
                            pg[:, nt * 512:(nt + 1) * 512],
                            kn8s[b][:, :, nj * 128:(nj + 1) * 128],
                            qn8s[b][:, :, nt * 512:(nt + 1) * 512],
                            start=True, stop=True, perf_mode=DR)

                def exp_chunk(nj):
                    e = epool.tile([128, HW], BF16, tag="e", name="e")
                    es[nj] = e
                    nc.scalar.activation(
                        e, pgs[nj], AF.Exp, scale=rkts[b][:, nj:nj + 1])

                def numer_chunk(nj):
                    for nt in range(2):
                        nc.tensor.matmul(
                            pnd[0:33, nt * 512:(nt + 1) * 512],
                            v1s[b][:, nj, :],
                            es[nj][:, nt * 512:(nt + 1) * 512],
                            start=(nj == 0), stop=(nj == NNJ - 1))

                gram_chunk(0)
                exp_chunk(0)
                for nj in range(1, NNJ):
                    gram_chunk(nj)
                    exp_chunk(nj)
                    numer_chunk(nj - 1)
                    if nj == 2 and mid_mm is not None:
                        mid_mm()
                numer_chunk(NNJ - 1)

            V1s = {}

            def vchain_dve(b):
                # V row = pnd[0] * exp(-ln(pnd[32]))
                lnd = sbuf.tile([1, HW], F32, tag="lnd", name="lnd")
                nc.scalar.activation(lnd, pnds[b][32:33, :], AF.Ln)
                rd = sbuf.tile([1, HW], F32, tag="rd", name="rd")
                nc.scalar.activation(rd, lnd, AF.Exp, scale=-1.0)
                V1 = sbuf.tile([1, HW], BF16, tag="V1", name="V1")
                V1s[b] = V1
                nc.vector.tensor_mul(V1, pnds[b][0:1, :], rd)

            def vchain_mm(b):
                V_bc = unp.tile([128, HW], F32, tag="un", name="V_bc")
                Vbcs[b] = V_bc
                for nt in range(2):
                    nc.tensor.matmul(
                        V_bc[:, nt * 512:(nt + 1) * 512], ones1,
                        V1s[b][:, nt * 512:(nt + 1) * 512], start=True, stop=True)

            def conv(b, vert_eng):
                # V_bc PSUM -> SBUF bf16 once, then all-bf16 DVE (+Pool when
                # overlapped with the other image's gram)
                V_sb = sbuf.tile([128, HW], BF16, tag="V_sb", name="V_sb")
                nc.vector.tensor_copy(V_sb, Vbcs[b])
                for mc in range(NCH):
                    p_sb = sbuf.tile([128, H, W], BF16, tag="p_sb", name="p_sb")
                    nc.vector.tensor_mul(
                        p_sb.rearrange("p h w -> p (h w)"), xs[b][:, mc, :], V_sb)
                    t1 = sbuf.tile([128, H, 31], BF16, tag="t1", name="t1")
                    nc.vector.tensor_add(t1, p_sb[:, :, 0:31], p_sb[:, :, 1:32])
                    s1 = sbuf.tile([128, H, CW], BF16, tag="s1", name="s1")
                    nc.vector.tensor_add(s1, t1[:, :, 0:CW], p_sb[:, :, 2:32])
                    o1 = sbuf.tile([128, CH, CW], BF16, tag="o1", name="o1")
                    vert_eng.tensor_add(o1, s1[:, 0:CH, :], s1[:, 1:CH + 1, :])
                    outb = sbuf.tile([128, NF], BF16, tag="outb", name="outb")
                    vert_eng.tensor_add(
                        outb.rearrange("p (h w) -> p h w", h=CH),
                        o1, s1[:, 2:CH + 2, :])
                    # casting DMA bf16 -> f32 (gpsimd SWDGE)
                    nc.gpsimd.dma_start(
                        out=out_d[b, mc * 128:(mc + 1) * 128, :], in_=outb)

            # ---------------- emission schedule ----------------
            load_x(0)
            load_rest_consts()
            load_x(1)
            warmup()
            proj(0)
            proj(1)
            gram(0)
            vchain_dve(0)
            gram(1, mid_mm=lambda: vchain_mm(0))
            conv(0, nc.gpsimd)
            vchain_dve(1)
            vchain_mm(1)
            conv(1, nc.vector)

    nc.compile()
    return nc


_CACHE = {}


def _get_program():
    if "nc" not in _CACHE:
        _CACHE["nc"] = build_program()
    return _CACHE["nc"]


def make_in_maps(batch, key_w, key_b, query_w, query_b, value_w, value_b):
    import ml_dtypes
    bf = ml_dtypes.bfloat16
    wall = np.zeros((C, 2 * C + 2), np.float32)
    wall[:, 0:C] = query_w.T
    wall[:, C:2 * C] = key_w.T
    wall[:, 2 * C] = value_w[0]
    wall = wall.astype(bf)
    ball = np.zeros((128, 2 * NCH), np.float32)
    ball[:, 0:NCH] = key_b.reshape(NCH, 128).T
    ball[:, NCH:2 * NCH] = query_b.reshape(NCH, 128).T
    bv = np.zeros((1, 2), np.float32)
    bv[0, 0] = value_b[0]
    in_maps = []
    for i in range(NCORES):
        xb = batch[i * BL:(i + 1) * BL].reshape(BL, C, HW).astype(bf)
        in_maps.append({
            "x": np.ascontiguousarray(xb),
            "wall": wall, "ball": ball, "bv": bv,
        })
    return in_maps


def kernel(batch, key_w, key_b, query_w, query_b, value_w, value_b,
           local_indices=None, **_ignored):
    batch = np.ascontiguousarray(np.asarray(batch, np.float32))
    args = [np.asarray(a, np.float32) for a in
            (key_w, key_b, query_w, query_b, value_w, value_b)]
    nc = _get_program()
    in_maps = make_in_maps(batch, *args)
    res = run_bass_kernel_spmd(nc, in_maps, list(range(NCORES)))
    outs = [np.asarray(r["out"], np.float32) for r in res.results]
    return np.concatenate(outs, axis=0).reshape(B, C, CH, CW)


# revision 14
# speedup vs baseline: 1.4414x; 1.0724x over previous
"""Trainium2 Bass kernel for nn_ConvolutionalSelfAttention.

The reference network collapses algebraically. Per image b (Xt = batch[b]
viewed [C, HW], c-major):
  K_t = key_w @ Xt + key_b          [C, HW]
  Q_t = query_w @ Xt + query_b      [C, HW]
  v   = value_w @ Xt + value_b      [HW]
  rk[n] = 1/||K_t[:,n]||, rq[m] = 1/||Q_t[:,m]||
  E[n,m] = exp(rk[n] rq[m] (K_t[:,n] . Q_t[:,m]))       (full 1024x1024 Gram)
  V[m] = (sum_n v[n] E[n,m]) / (sum_n E[n,m])
  P[c,m] = Xt[c,m] V[m]
  out[b,c,i,j] = 3x3 valid box-sum of P over the spatial grid

Sharding: data-parallel over batch (16 images over 8 cores, 2 each).

v2 design:
- host ships x / weights in bf16 (half the input DMA, feeds bf16 PE ops)
- Gram in fp8e4 with MatmulPerfMode.DoubleRow (0.5 cyc/col, K=256 one pass)
- exp on ACT -> es bf16; numer/denom via one [v;1]-stationary matmul chunk
- rq/V broadcast via PE ones-matmul; qn8 = (ppQ+bq)*rq_bc fused on DVE
- 3x3 box-sum fully on DVE/Pool in bf16 (DVE 4x mode), output written with
  a casting gpsimd DMA (bf16 -> f32), freeing PE + PSUM in the tail
- rk / v transposed to partition form via small DRAM bounces (off critical
  path)
"""
import os
import numpy as np

os.environ.setdefault("BASS_NEVER_TRACE", "1")

import contextlib

import concourse.bass as bass
import concourse.bacc as bacc
import concourse.tile as tile
from concourse import mybir
from concourse.bass_utils import run_bass_kernel_spmd

F32 = mybir.dt.float32
F32R = mybir.dt.float32r
BF16 = mybir.dt.bfloat16
F8 = mybir.dt.float8e4
AF = mybir.ActivationFunctionType
OP = mybir.AluOpType
DR = mybir.MatmulPerfMode.DoubleRow

B, C, H, W = 16, 256, 32, 32
HW = H * W            # 1024
CH = CW = 30
NF = CH * CW          # 900
NCORES = 8
BL = B // NCORES      # images per core
NCH = C // 128        # channel chunks
NNJ = HW // 128       # row blocks of the gram


def _ap(t, extra_off, pattern):
    return bass.AP(tensor=t.tensor, offset=t.offset + extra_off,
                   ap=[list(x) for x in pattern])


def build_program():
    nc = bacc.Bacc("TRN2", target_bir_lowering=False, debug=False,
                   num_devices=NCORES)
    # walrus's lower_act places activation-table loads; bacc's pre-placed
    # loads produce NEFFs this runtime refuses to load.
    nc.insert_act_table_loads = lambda: None

    def din(name, shape, dt):
        return nc.dram_tensor(name, list(shape), dt, kind="ExternalInput").ap()

    x_d = din("x", (BL, C, HW), BF16)
    wall_d = din("wall", (C, 2 * C + 2), BF16)  # [query_w.T | key_w.T | wv | 0]
    ball_d = din("ball", (128, 2 * NCH), F32)   # [bk | bq]
    bv_d = din("bv", (1, 2), F32)

    out_d = nc.dram_tensor("out", [BL, C, NF], F32, kind="ExternalOutput").ap()

    with tile.TileContext(nc) as tc:
        with contextlib.ExitStack() as ctx:
            consts = ctx.enter_context(tc.tile_pool(name="consts", bufs=1))
            sbuf = ctx.enter_context(tc.tile_pool(name="sbuf", bufs=2))
            epool = ctx.enter_context(tc.tile_pool(name="epool", bufs=4))
            pgp = ctx.enter_context(tc.tile_pool(name="pgp", bufs=2, space="PSUM"))
            unp = ctx.enter_context(tc.tile_pool(name="unp", bufs=2, space="PSUM"))
            dramp = ctx.enter_context(tc.tile_pool(name="dramp", bufs=2, space="DRAM"))

            # ---------------- constants ----------------
            wall_t = consts.tile([128, NCH, 2 * C + 2], BF16, tag="wall", name="wall")
            wallv = wall_d.rearrange("(kc p) m -> p kc m", p=128)
            nc.sync.dma_start(out=wall_t[:, :, 0:C], in_=wallv[:, :, 0:C])
            wq_t = wall_t[:, :, 0:C]
            wk_t = wall_t[:, :, C:2 * C]
            wv_t = wall_t[:, :, 2 * C:2 * C + 1]
            ball_t = consts.tile([128, 2 * NCH], F32, tag="ball", name="ball")
            bk_t = ball_t[:, 0:NCH]
            bq_t = ball_t[:, NCH:2 * NCH]
            bv_t = consts.tile([128, 1], F32, tag="bv", name="bv")
            ones_b = consts.tile([128, 1], BF16, tag="ones_b", name="ones_b")
            nc.vector.memset(ones_b, 1.0)
            ones1 = consts.tile([1, 128], BF16, tag="ones1", name="ones1")
            nc.vector.memset(ones1, 1.0)
            lnc = consts.tile([1, 2], F32, tag="lnc", name="lnc")
            nc.vector.memset(lnc[:, 0:1], float(np.log(16.0)))
            nc.vector.memset(lnc[:, 1:2], float(-np.log(16.0)))

            def load_rest_consts():
                nc.scalar.dma_start(out=wall_t[:, :, C:], in_=wallv[:, :, C:])
                nc.sync.dma_start(out=ball_t, in_=ball_d)
                nc.sync.dma_start(out=bv_t, in_=_ap(bv_d, 0, [[0, 128], [1, 1]]))

            def warmup():
                pw = pgp.tile([128, HW], F32, tag="pg", name="warm")
                for _ in range(30):
                    nc.tensor.matmul(pw[0:1, 0:1], ones_b, ones_b,
                                     start=True, stop=True)

            # ---------------- per-b state ----------------
            xs, kn8s, qn8s, rkts, v1s = {}, {}, {}, {}, {}
            pnds, Vbcs = {}, {}

            def load_x(b):
                xs[b] = sbuf.tile([128, NCH, HW], BF16, tag="x", name="x")
                xv = x_d[b].rearrange("(kc p) m -> p kc m", p=128)
                engs = [nc.sync, nc.scalar]
                for kc in range(NCH):
                    engs[kc].dma_start(out=xs[b][:, kc, :], in_=xv[:, kc, :])

            def mm_proj(psum, w_t, mc, b):
                for kc in range(NCH):
                    for nt in range(2):
                        nc.tensor.matmul(
                            psum[:, nt * 512:(nt + 1) * 512],
                            w_t[:, kc, mc * 128:(mc + 1) * 128],
                            xs[b][:, kc, nt * 512:(nt + 1) * 512],
                            start=(kc == 0), stop=(kc == NCH - 1))

            def proj(b):
                # ---- v projection first (pgp pool, frees early) ----
                pv = pgp.tile([128, HW], F32, tag="pg", name="pv")
                for nt in range(2):
                    for kc in range(NCH):
                        nc.tensor.matmul(
                            pv[0:1, nt * 512:(nt + 1) * 512], wv_t[:, kc, :],
                            xs[b][:, kc, nt * 512:(nt + 1) * 512],
                            start=(kc == 0), stop=(kc == NCH - 1))
                v_sb = sbuf.tile([1, HW], F32, tag="v_sb", name="v_sb")
                nc.vector.tensor_scalar_add(v_sb, pv[0:1, :], bv_t[0:1, 0:1])
                d_v = dramp.tile([1, HW], F32, tag="d_v", name="d_v")
                nc.sync.dma_start(out=d_v, in_=v_sb)
                vt = sbuf.tile([128, NNJ], F32, tag="vt", name="vt")
                nc.sync.dma_start(out=vt, in_=_ap(d_v, 0, [[1, 128], [128, NNJ]]))
                v1 = sbuf.tile([128, NNJ, 33], BF16, tag="v1", name="v1")
                v1s[b] = v1
                nc.gpsimd.memset(v1, 0.0)
                nc.gpsimd.memset(v1[:, :, 32], 1.0)
                nc.gpsimd.tensor_copy(v1[:, :, 0], vt)

                # ---- Q projections (pgp pool) ----
                sqq = sbuf.tile([128, NCH, HW], BF16, tag="sqq", name="sqq")
                ppqs = []
                for mc in range(NCH):
                    pp = pgp.tile([128, HW], F32, tag="pg", name="ppq")
                    ppqs.append(pp)
                    mm_proj(pp, wq_t, mc, b)
                    # sqQ = (pp + bq)^2 on ACT -> fp8
                    nc.scalar.activation(
                        sqq[:, mc, :], pp, AF.Square, bias=bq_t[:, mc:mc + 1])

                # ---- K projections (unp pool: no rq-chain aliasing) ----
                kn8 = sbuf.tile([128, NCH, HW], F8, tag="kn8", name="kn8")
                kn8s[b] = kn8
                sqk = sbuf.tile([128, NCH, HW], BF16, tag="sqk", name="sqk")
                for mc in range(NCH):
                    pp = unp.tile([128, HW], F32, tag="un", name="ppk")
                    mm_proj(pp, wk_t, mc, b)
                    nc.vector.tensor_scalar_add(kn8[:, mc, :], pp, bk_t[:, mc:mc + 1])
                    nc.vector.tensor_mul(sqk[:, mc, :], kn8[:, mc, :], kn8[:, mc, :])

                # ---- sum-of-squares (rows 0 / 32) ----
                pssq = unp.tile([128, HW], F32, tag="un", name="pssq")
                for nt in range(2):
                    for kc in range(NCH):
                        nc.tensor.matmul(
                            pssq[0:1, nt * 512:(nt + 1) * 512], ones_b,
                            sqq[:, kc, nt * 512:(nt + 1) * 512],
                            start=(kc == 0), stop=(kc == NCH - 1))
                    for kc in range(NCH):
                        nc.tensor.matmul(
                            pssq[32:33, nt * 512:(nt + 1) * 512], ones_b,
                            sqk[:, kc, nt * 512:(nt + 1) * 512],
                            start=(kc == 0), stop=(kc == NCH - 1))

                # rq chain: exp(-0.5*ln(ssq) + ln16) = 16/||Q|| (ACT only)
                rr = sbuf.tile([1, HW], F32, tag="rr", name="rr")
                nc.scalar.activation(rr, pssq[0:1, :], AF.Ln)
                rq1 = sbuf.tile([1, HW], BF16, tag="rq1", name="rq1")
                nc.scalar.activation(rq1, rr, AF.Exp, scale=-0.5,
                                     bias=lnc[0:1, 0:1])
                rq_bc = unp.tile([128, HW], F32, tag="un", name="rq_bc")
                for nt in range(2):
                    nc.tensor.matmul(
                        rq_bc[:, nt * 512:(nt + 1) * 512], ones1,
                        rq1[:, nt * 512:(nt + 1) * 512], start=True, stop=True)

                # rk chain: exp(-0.5*ln(ssq) - ln16) = rk/16, bounce to [128, NNJ]
                rrk = sbuf.tile([1, HW], F32, tag="rrk", name="rrk")
                nc.scalar.activation(rrk, pssq[32:33, :], AF.Ln)
                rk_row = sbuf.tile([1, HW], F32, tag="rk_row", name="rk_row")
                nc.scalar.activation(rk_row, rrk, AF.Exp, scale=-0.5,
                                     bias=lnc[0:1, 1:2])
                d_rk = dramp.tile([1, HW], F32, tag="d_rk", name="d_rk")
                nc.sync.dma_start(out=d_rk, in_=rk_row)
                rkt = sbuf.tile([128, NNJ], F32, tag="rkt", name="rkt")
                rkts[b] = rkt
                nc.sync.dma_start(out=rkt, in_=_ap(d_rk, 0, [[1, 128], [128, NNJ]]))

                # ---- qn8 = (ppQ + bq) * rq_sb -> fp8 (fused on DVE) ----
                rq_sb = sbuf.tile([128, HW], BF16, tag="rq_sb", name="rq_sb")
                nc.vector.tensor_copy(rq_sb, rq_bc)
                qn8 = sbuf.tile([128, NCH, HW], F8, tag="qn8", name="qn8")
                qn8s[b] = qn8
                for mc in range(NCH):
                    nc.vector.scalar_tensor_tensor(
                        qn8[:, mc, :], ppqs[mc], bq_t[:, mc:mc + 1], rq_sb,
                        op0=OP.add, op1=OP.mult)

            def gram(b, mid_mm=None):
                pnd = unp.tile([128, HW], F32, tag="un", name="pnd")
                pnds[b] = pnd
                pgs, es = [None] * NNJ, [None] * NNJ

                def gram_chunk(nj):
                    pg = pgp.tile([128, HW], F32, tag="pg", name="pg")
                    pgs[nj] = pg
                    for nt in range(2):
                        nc.tensor.matmul(
                            pg[:, nt * 512:(nt + 1) * 512],
                            kn8s[b][:, :, nj * 128:(nj + 1) * 128],
                            qn8s[b][:, :, nt * 512:(nt + 1) * 512],
                            start=True, stop=True, perf_mode=DR)

                def exp_chunk(nj):
                    e = epool.tile([128, HW], BF16, tag="e", name="e")
                    es[nj] = e
                    nc.scalar.activation(
                        e, pgs[nj], AF.Exp, scale=rkts[b][:, nj:nj + 1])

                def numer_chunk(nj):
                    for nt in range(2):
                        nc.tensor.matmul(
                            pnd[0:33, nt * 512:(nt + 1) * 512],
                            v1s[b][:, nj, :],
                            es[nj][:, nt * 512:(nt + 1) * 512],
                            start=(nj == 0), stop=(nj == NNJ - 1))

                gram_chunk(0)
                exp_chunk(0)
                for nj in range(1, NNJ):
                    gram_chunk(nj)
                    exp_chunk(nj)
                    numer_chunk(nj - 1)
                    if nj == 2 and mid_mm is not None:
                        mid_mm()
                numer_chunk(NNJ - 1)

            V1s = {}

            def vchain_dve(b):
                # V row = pnd[0] * exp(-ln(pnd[32]))
                lnd = sbuf.tile([1, HW], F32, tag="lnd", name="lnd")
                nc.scalar.activation(lnd, pnds[b][32:33, :], AF.Ln)
                rd = sbuf.tile([1, HW], F32, tag="rd", name="rd")
                nc.scalar.activation(rd, lnd, AF.Exp, scale=-1.0)
                V1 = sbuf.tile([1, HW], BF16, tag="V1", name="V1")
                V1s[b] = V1
                nc.vector.tensor_mul(V1, pnds[b][0:1, :], rd)

            def vchain_mm(b):
                V_bc = unp.tile([128, HW], F32, tag="un", name="V_bc")
                Vbcs[b] = V_bc
                for nt in range(2):
                    nc.tensor.matmul(
                        V_bc[:, nt * 512:(nt + 1) * 512], ones1,
                        V1s[b][:, nt * 512:(nt + 1) * 512], start=True, stop=True)

            def conv(b, vert_eng):
                for mc in range(NCH):
                    p_sb = sbuf.tile([128, H, W], BF16, tag="p_sb", name="p_sb")
                    nc.vector.tensor_mul(
                        p_sb.rearrange("p h w -> p (h w)"), xs[b][:, mc, :],
                        Vbcs[b])
                    t1 = sbuf.tile([128, H, 31], BF16, tag="t1", name="t1")
                    nc.vector.tensor_add(t1, p_sb[:, :, 0:31], p_sb[:, :, 1:32])
                    s1 = sbuf.tile([128, H, CW], BF16, tag="s1", name="s1")
                    nc.vector.tensor_add(s1, t1[:, :, 0:CW], p_sb[:, :, 2:32])
                    o1 = sbuf.tile([128, CH, CW], BF16, tag="o1", name="o1")
                    vert_eng.tensor_add(o1, s1[:, 0:CH, :], s1[:, 1:CH + 1, :])
                    outb = sbuf.tile([128, NF], BF16, tag="outb", name="outb")
                    vert_eng.tensor_add(
                        outb.rearrange("p (h w) -> p h w", h=CH),
                        o1, s1[:, 2:CH + 2, :])
                    # casting DMA bf16 -> f32 (gpsimd SWDGE)
                    nc.gpsimd.dma_start(
                        out=out_d[b, mc * 128:(mc + 1) * 128, :], in_=outb)

            # ---------------- emission schedule ----------------
            load_x(0)
            load_rest_consts()
            load_x(1)
            warmup()
            proj(0)
            proj(1)
            gram(0)
            vchain_dve(0)
            gram(1, mid_mm=lambda: vchain_mm(0))
            conv(0, nc.gpsimd)
            vchain_dve(1)
            vchain_mm(1)
            conv(1, nc.vector)

    nc.compile()
    return nc


_CACHE = {}


def _get_program():
    if "nc" not in _CACHE:
        _CACHE["nc"] = build_program()
    return _CACHE["nc"]


def make_in_maps(batch, key_w, key_b, query_w, query_b, value_w, value_b):
    import ml_dtypes
    bf = ml_dtypes.bfloat16
    wall = np.zeros((C, 2 * C + 2), np.float32)
    wall[:, 0:C] = query_w.T
    wall[:, C:2 * C] = key_w.T
    wall[:, 2 * C] = value_w[0]
    wall = wall.astype(bf)
    ball = np.zeros((128, 2 * NCH), np.float32)
    ball[:, 0:NCH] = key_b.reshape(NCH, 128).T
    ball[:, NCH:2 * NCH] = query_b.reshape(NCH, 128).T
    bv = np.zeros((1, 2), np.float32)
    bv[0, 0] = value_b[0]
    in_maps = []
    for i in range(NCORES):
        xb = batch[i * BL:(i + 1) * BL].reshape(BL, C, HW).astype(bf)
        in_maps.append({
            "x": np.ascontiguousarray(xb),
            "wall": wall, "ball": ball, "bv": bv,
        })
    return in_maps


def kernel(batch, key_w, key_b, query_w, query_b, value_w, value_b,
           local_indices=None, **_ignored):
    batch = np.ascontiguousarray(np.asarray(batch, np.float32))
    args = [np.asarray(a, np.float32) for a in
            (key_w, key_b, query_w, query_b, value_w, value_b)]
    nc = _get_program()
    in_maps = make_in_maps(batch, *args)
    res = run_bass_kernel_spmd(nc, in_maps, list(range(NCORES)))
    outs = [np.asarray(r["out"], np.float32) for r in res.results]
    return np.concatenate(outs, axis=0).reshape(B, C, CH, CW)
